# revision 18
# baseline (speedup 1.0000x reference)
"""Trainium2 Bass kernel for nn_BlockSelfAttentionModule.

Reference semantics (B=4, H=8, L=1024, I=16 instruments, F=64 frames, D=64):
  out[b*H+h, l, m] = q[l] . r_instrument[l%I, m%I, :, h]
                   + q[l] . a_h[(l//I - m//I) mod (F+1)]
  where a_h = concat(e_past[:, :, h], -111 pad row)   # (65, D)

Both bias terms factor through small per-row tables:
  Ui[l, c]  = q[l] . R_h[l%I, c]           (L x 16)
  Psh[l, f] = q[l] . a_h[(l//I - f) % 65]  (L x 64)
  out[l, f*16 + c] = Psh[l, f] + Ui[l, c]

Strategy (8 cores data-parallel over the 32 = B*H rows, 4 rows/core;
all-bf16, rel-err ~5e-3 against the 2e-2 gate):
  * bf16 output halves the dominant HBM write (16.8 -> 8.4 MiB/core).
    DMA transfers serialize on the ISSUING engine (~332 GB/s each, cost
    ~ free-bytes, partition-count-free), so per-tile output DMAs are
    spread across SP, Activation, and Pool(SWDGE) for ~1 TB/s aggregate.
  * Host pre-builds per (b*H+h) row: the block-diagonal zero-padded zq
    (so K=128 matmuls cover 2 frames at once), a packed qT|rt table, and
    the shared reversed-diagonal time table a2.
  * PE: 16 ui matmuls -> Ui^T in one batched PSUM tile; 16 psh matmuls
    per half-row into one batched PSUM tile; 8 transposes of Ui^T into
    one PSUM tile. All matmul operands start at partition 0 (operands at
    partition offset 64 crash the exec unit) and GPSIMD never touches
    PSUM (illegal) — PSUM evacuation is batched on DVE: one copy per
    half-row doubles Psh into pairs (p2sb[l, 2f+c2]) and one copy per
    row moves the transposed Ui, minimizing per-op PSUM access cost.
  * The full-size expansion out[l, (f, c-hi, c2)] = psh2 + ui runs as
    tensor_tensor adds whose operands are all 2-byte packed (last dim
    count 2), hitting DVE's 2x mode (594 ns/tile); a tuned share of
    tiles runs on the otherwise-idle Pool engine (853 ns/tile), with
    per-tile DMA engine assignment balancing all five engines.
Host casts the returned bf16 (4, L, L) blocks back to f32.
"""

import numpy as np
import ml_dtypes

import concourse.bass as bass
import concourse.bacc as bacc
import concourse.mybir as mybir
from concourse import masks
from concourse.tile import TileContext
from concourse.bass_utils import run_bass_kernel_spmd

F32 = mybir.dt.float32
BF16 = mybir.dt.bfloat16
NP_BF16 = ml_dtypes.bfloat16

N_CORES = 8
ROWS_PER_CORE = 4  # (b*H + h) rows per core
L = 1024
D = 64
I = 16
F = 64
PAD_VAL = -111.0

IT_COLS = 1024 + 256  # qT | rt

# Per-tile assignment tables, tuned against CoreSim.
# tt engine: which engine runs the expansion add for global tile t (0..31).
# dma engine: which engine issues the output DMA for tile t.
DEFAULT_TT = ["pool" if _t % 16 in (1, 3, 5, 7, 9, 11, 13) else "dve"
              for _t in range(32)]
DEFAULT_TT[31] = "pool"

# out-DMA engines: alternate SP/ACT, two mid-stream Pool DMAs
DEFAULT_DMA = ["sync" if _t % 2 == 0 else "scalar" for _t in range(32)]
for _t in (9, 19, 29):
    DEFAULT_DMA[_t] = "gpsimd"

_PROGRAM = None


def build_row(nc, pools, j, tt_map, dma_map, uit_eng):
    (qrtd, a2all, out) = pools["dram"]
    ident = pools["ident"]
    (qrtp, uitraw, p2sbp, uisbp, otp) = pools["sbuf"]
    (psui, ps2p, uptp) = pools["psum"]

    it = pools["loaded"][j]
    a2 = pools["a2t"][:, j * 128 : (j + 1) * 128]
    zq = pools["zq"][j]

    # --- Ui^T: uitr[c, f*16+i] = q[f*16+i] . R_h[i, c]  (K=64) ---
    uitr = uitraw.tile([I, L], BF16)
    qv = it[:, 0:1024].rearrange("d (f i) -> d i f", i=I)
    for hi in range(2):  # instrument half
        pst = psui.tile([I, 512], F32)
        for i8 in range(8):
            i = hi * 8 + i8
            nc.tensor.matmul(
                pst[:, i8 * 64 : (i8 + 1) * 64],
                it[:, 1024 + I * i : 1024 + I * (i + 1)],
                qv[:, i, :],
            )
        ov = uitr[:].rearrange("c (f i) -> c i f", i=I)[
            :, hi * 8 : (hi + 1) * 8, :
        ]
        iv = pst[:].rearrange("c (i8 f) -> c i8 f", f=64)
        eng = uit_eng if isinstance(uit_eng, str) else uit_eng[hi]
        copy = nc.scalar.copy if eng == "scalar" else nc.vector.tensor_copy
        if j == 0:
            # row 0: copy f<8 first so tile 0's transpose unblocks early
            copy(ov[:, :, 0:8], iv[:, :, 0:8])
            copy(ov[:, :, 8:64], iv[:, :, 8:64])
        else:
            copy(ov, iv)

    # --- psh for the first half's 4 tiles (PE work emitted before the
    # transposes so PE never stalls waiting for the uiT copies) ---
    ps2_halves = []
    ps2 = ps2p.tile([128, 256], F32, name="ps2")
    for tq in range(4):
        lt = tq
        for g in range(4):
            fp = lt * 4 + g
            nc.tensor.matmul(
                ps2[g * 32 : (g + 1) * 32, tq * 64 : (tq + 1) * 64],
                zq[:, fp * 32 : (fp + 1) * 32],
                a2[:, 64 - 2 * fp : 128 - 2 * fp],
                tile_position=(0, g * 32),
            )
    ps2_halves.append(ps2)

    # --- ui, l-partitioned: 8 transposes into one PSUM tile, one evac ---
    upt = uptp.tile([128, 128], BF16)
    for lt in range(8):
        nc.tensor.transpose(
            upt[:, lt * I : (lt + 1) * I],
            uitr[:, lt * 128 : (lt + 1) * 128],
            ident[:],
        )
    uisb = uisbp.tile([128, 128], BF16)
    nc.vector.tensor_copy(uisb[:], upt[:])

    for half in range(2):
        if half == 1:
            ps2 = ps2p.tile([128, 256], F32, name="ps2")
            for tq in range(4):
                lt = 4 + tq
                for g in range(4):
                    fp = lt * 4 + g
                    nc.tensor.matmul(
                        ps2[g * 32 : (g + 1) * 32, tq * 64 : (tq + 1) * 64],
                        zq[:, fp * 32 : (fp + 1) * 32],
                        a2[:, 64 - 2 * fp : 128 - 2 * fp],
                        tile_position=(0, g * 32),
                    )
        else:
            ps2 = ps2_halves[0]
        # p2sb[p, 128*tq + 2f + c2] = psh pairs, bf16
        p2sb = p2sbp.tile([128, 512], BF16)
        p2v = p2sb[:].rearrange("p (tq f c2) -> p tq f c2", tq=4, c2=2)
        p2src = (
            ps2[:].rearrange("p (tq f) -> p tq f", tq=4)
            .unsqueeze(3).broadcast_to([128, 4, 64, 2])
        )
        if j == 0 and half == 0:
            # row 0: evacuate tile 0's psh first so its tt unblocks early
            nc.vector.tensor_copy(p2v[:, 0:1], p2src[:, 0:1])
            nc.vector.tensor_copy(p2v[:, 1:4], p2src[:, 1:4])
        else:
            nc.vector.tensor_copy(p2v, p2src)

        for tq in range(4):
            lt = half * 4 + tq
            t = j * 8 + lt
            # ot[p, f*16 + 2*ch + c2] = psh2[p, 2f+c2] + ui[p, 2ch+c2]
            ot = otp.tile([128, L], BF16)
            ov = ot[:].rearrange("p (f ch c2) -> p f ch c2", ch=8, c2=2)
            in1 = (
                uisb[:, lt * I : (lt + 1) * I]
                .rearrange("p (ch c2) -> p ch c2", c2=2)
                .unsqueeze(1)
                .broadcast_to([128, 64, 8, 2])
            )
            in0 = (
                p2sb[:, tq * 128 : (tq + 1) * 128]
                .rearrange("p (f c2) -> p f c2", c2=2)
                .unsqueeze(2)
                .broadcast_to([128, 64, 8, 2])
            )
            if tt_map[t] == "pool":
                nc.gpsimd.tensor_tensor(ov, in0, in1, mybir.AluOpType.add)
            else:
                nc.vector.tensor_tensor(ov, in0, in1, mybir.AluOpType.add)

            dst = out[j].rearrange("(t p) m -> p t m", p=128)[:, lt, :]
            getattr(nc, dma_map[t]).dma_start(dst, ot[:])


def build_program(loop_iters: int | None = None, tt_map=None, dma_map=None,
                  uit_eng=("scalar", "scalar"), obufs: int = 10) -> bass.Bass:
    """loop_iters: device-side repeat loop for benchmarking only."""
    tt_map = tt_map or DEFAULT_TT
    dma_map = dma_map or DEFAULT_DMA
    nc = bacc.Bacc("TRN2", debug=False, num_devices=N_CORES)
    qrtd = nc.declare_dram_parameter(
        "qrt", [ROWS_PER_CORE, D, IT_COLS], BF16, isOutput=False
    )
    zqd = nc.declare_dram_parameter(
        "zqd", [ROWS_PER_CORE, 128, L], BF16, isOutput=False
    )
    a2all = nc.declare_dram_parameter(
        "a2all", [128, ROWS_PER_CORE * 128], BF16, isOutput=False
    )
    out = nc.declare_dram_parameter(
        "out", [ROWS_PER_CORE, L, L], BF16, isOutput=True
    )

    with TileContext(nc) as tc:
        with (
            tc.tile_pool(name="const", bufs=1) as constp,
            tc.tile_pool(name="qrtp", bufs=ROWS_PER_CORE) as qrtp,
            tc.tile_pool(name="uitraw", bufs=2) as uitraw,
            tc.tile_pool(name="p2sb", bufs=3) as p2sbp,
            tc.tile_pool(name="uisb", bufs=2) as uisbp,
            tc.tile_pool(name="otp", bufs=obufs) as otp,
            tc.tile_pool(name="psui", bufs=2, space="PSUM") as psui,
            tc.tile_pool(name="ps2", bufs=3, space="PSUM") as ps2p,
            tc.tile_pool(name="upt", bufs=2, space="PSUM") as uptp,
        ):
            ident = constp.tile([I, I], BF16)
            masks.make_identity(nc, ident[:])
            a2t = constp.tile([128, ROWS_PER_CORE * 128], BF16)
            zqs = [constp.tile([128, L], BF16, name=f"zqt{j}")
                   for j in range(ROWS_PER_CORE)]

            pools = {
                "dram": (qrtd, a2all, out),
                "sbuf": (qrtp, uitraw, p2sbp, uisbp, otp),
                "psum": (psui, ps2p, uptp),
                "ident": ident,
                "a2t": a2t,
                "zq": zqs,
            }

            def body(_iv=None):
                pools["loaded"] = []
                nc.scalar.dma_start(a2t[:], a2all[:])
                for j in range(ROWS_PER_CORE):
                    eng = nc.sync if j < 3 else nc.scalar
                    it = qrtp.tile([D, IT_COLS], BF16, name=f"it{j}")
                    # qrt first: the ui chain (row-critical) needs it before
                    # the psh matmuls need zq
                    eng.dma_start(it[:], qrtd[j])
                    eng.dma_start(zqs[j][:], zqd[j])
                    pools["loaded"].append(it)
                for j in range(ROWS_PER_CORE):
                    build_row(nc, pools, j, tt_map, dma_map, uit_eng)

            if loop_iters is None:
                body()
            else:
                with tc.For_i(0, loop_iters, 1) as _iv:
                    body(_iv)
    return nc


def make_in_maps(q, r_instrument, e_past):
    """Host-side sharding + table prep. Returns per-core input dicts."""
    q = np.asarray(q, dtype=np.float32)
    r_instrument = np.asarray(r_instrument, dtype=np.float32)
    e_past = np.asarray(e_past, dtype=np.float32)

    qT = q.reshape(32, L, D).transpose(0, 2, 1)  # (32, D, L)

    # zq[r, s*64+d, l] = qT[r, d, l] where (l//16) % 2 == s, else 0
    par = (np.arange(L) // I) % 2
    zq = np.zeros((32, 128, L), np.float32)
    for s in (0, 1):
        cols = par == s
        zq[:, s * 64 : (s + 1) * 64, cols] = qT[:, :, cols]

    # rt[h, d, 16i+c] = R[i, c, d, h]
    rt = r_instrument.transpose(3, 2, 0, 1).reshape(8, D, I * I)  # (8, D, 256)

    # a2[h, s*64+d, t] = a_h[(64 - t + s) % 65, d], t in [0, 128)
    a = np.concatenate(
        [e_past, np.full((1, D, 8), PAD_VAL, dtype=np.float32)], axis=0
    )  # (65, D, H)
    idx2 = (64 - np.arange(128)[None, :] + np.arange(2)[:, None]) % 65
    a2 = a[idx2]  # (2, 128, D, 8)
    a2 = a2.transpose(3, 0, 2, 1).reshape(8, 128, 128)  # (h, s*64+d, t)

    in_maps = []
    for k in range(N_CORES):
        rows = [ROWS_PER_CORE * k + j for j in range(ROWS_PER_CORE)]
        hs = [r % 8 for r in rows]
        # it[d] = [qT[d, :] | rt[d, :]]
        its = []
        for r, h in zip(rows, hs):
            its.append(
                np.concatenate([qT[r], rt[h]], axis=1)[None]
            )  # (1, 64, 1280)
        a2c = np.concatenate([a2[h] for h in hs], axis=1)  # (128, 512)
        in_maps.append(
            {
                "qrt": np.ascontiguousarray(
                    np.concatenate(its, axis=0).astype(NP_BF16)
                ),
                "a2all": np.ascontiguousarray(a2c.astype(NP_BF16)),
                "zqd": np.ascontiguousarray(zq[rows].astype(NP_BF16)),
            }
        )
    return in_maps


def _get_program() -> bass.Bass:
    global _PROGRAM
    if _PROGRAM is None:
        _PROGRAM = build_program()
        if not _PROGRAM.is_finalized():
            _PROGRAM.finalize()
    return _PROGRAM


def kernel(q, r_instrument, e_past, flipped_masks=None, **_unused):
    in_maps = make_in_maps(q, r_instrument, e_past)
    res = run_bass_kernel_spmd(_get_program(), in_maps, list(range(N_CORES))).results
    out = np.concatenate(
        [np.asarray(res[k]["out"], dtype=np.float32) for k in range(N_CORES)],
        axis=0,
    )
    return out.reshape(N_CORES * ROWS_PER_CORE, L, L)



# revision 19
# speedup vs baseline: 1.0949x; 1.0949x over previous
"""Trainium2 Bass kernel for nn_BlockSelfAttentionModule.

Reference semantics (B=4, H=8, L=1024, I=16 instruments, F=64 frames, D=64):
  out[b*H+h, l, m] = q[l] . r_instrument[l%I, m%I, :, h]
                   + q[l] . a_h[(l//I - m//I) mod (F+1)]
  where a_h = concat(e_past[:, :, h], -111 pad row)   # (65, D)

Both bias terms factor through small per-row tables:
  Ui[l, c]  = q[l] . R_h[l%I, c]           (L x 16)
  Psh[l, f] = q[l] . a_h[(l//I - f) % 65]  (L x 64)
  out[l, f*16 + c] = Psh[l, f] + Ui[l, c]

Strategy (8 cores data-parallel over the 32 = B*H rows, 4 rows/core):
  The rank-2 outer-sum structure means the full (L, L) block never needs
  the classic materialize-and-store path (on-chip expansion + 25 us of
  SBUF->DRAM writes). Each core:
    1. computes Ui f-partitioned (16 PE matmuls, stationary = q columns of
       one instrument) and Psh l-partitioned (32 PE matmuls via the
       zero-padded zq / skewed a2 tables, K=128 covering 2 frames), with
       one DVE PSUM evacuation each,
    2. stores tiny staging tables to DRAM: uis2 (4L x 16 x 2, pair-doubled
       Ui) and pshs (4L x 64), then pshs2 (4L x 2 x 64, row-doubled via
       one broadcast DMA per row),
    3. expands them into the full c-major output (out[l, c*64+f]) with
       DRAM->DRAM DMAs whose DRAM-side dim0 is the whole l*c axis, so each
       costs ~500 ns regardless of bytes:
         - uis2 -> uis8 broadcast cascade (x4), then 16 octet-position
           writes out[l, c*64+8o+k] = uis8[l, c, k]  (HWDGE, big desc
           counts are fine on SP/ACT),
         - 8 half-row gpsimd CCE-add accumulates out[...] += psh, reading
           pshs2's 128-element doubled runs so each stays at 4096 SWDGE
           descriptors (8192+ wedges the device ring).
  All sources keep a real contiguous fastest dim (stride-0 only in middle
  dims) and all lowered AP dim counts stay under the 16-bit ISA field
  limit - both are hard neuronxcc requirements.
Host casts the returned bf16 blocks to f32 and permutes the last axis
back from c-major to the reference n = f*16 + c order.
"""

import numpy as np
import ml_dtypes

import concourse.bass as bass
import concourse.bacc as bacc
import concourse.mybir as mybir

from concourse.tile import TileContext
from concourse.bass_utils import run_bass_kernel_spmd

F32 = mybir.dt.float32
BF16 = mybir.dt.bfloat16
NP_BF16 = ml_dtypes.bfloat16

N_CORES = 8
ROWS_PER_CORE = 4  # (b*H + h) rows per core
L = 1024
D = 64
I = 16
F = 64
PAD_VAL = -111.0

IT_COLS = 1024 + 256  # qT | rt

_PROGRAM = None

# Schedule config, tuned against CoreSim: input-load engine assignment and
# the emission order of per-row compute, staging stores, and expansions.
DEFAULT_CFG = {
    "loads": [
        ("a2", "pool"),
        ("it0", "sp"), ("zq0", "act"),
        ("it1", "sp"), ("zq1", "act"),
        ("it2", "sp"), ("zq2", "act"),
        ("it3", "sp"), ("zq3", "act"),
    ],
    "emit": (
        [("ui", 0), ("psh", 0), ("ui", 1), ("psh", 1),
         ("ui", 2), ("psh", 2), ("ui", 3), ("psh", 3),
         ("uis", 0, "act"), ("casc", 0, "sp"), ("casc", 1, "act"),
         ("pshs", 0, "sp"), ("dup", 0, "sp"),
         ("uis", 1, "act"), ("casc", 2, "sp"), ("casc", 3, "act"),
         ("pshs", 1, "sp"), ("dup", 1, "sp")]
        + [("oct", 0, 2, o, ("sp", "act")[o % 2]) for o in range(8)]
        + [("acc", 0), ("acc", 1), ("acc", 2), ("acc", 3)]
        + [("uis", 2, "act"), ("casc", 4, "sp"), ("casc", 5, "act"),
           ("pshs", 2, "sp"), ("dup", 2, "sp"),
           ("uis", 3, "act"), ("casc", 6, "sp"), ("casc", 7, "act"),
           ("pshs", 3, "sp"), ("dup", 3, "sp")]
        + [("oct", 2, 2, o, ("sp", "act")[o % 2]) for o in range(8)]
        + [("acc", 4), ("acc", 5), ("acc", 6), ("acc", 7)]
    ),
}


def build_program(loop_iters: int | None = None, cfg=None) -> bass.Bass:
    """loop_iters: device-side repeat loop for benchmarking only."""
    cfg = cfg or DEFAULT_CFG
    nc = bacc.Bacc("TRN2", debug=False, num_devices=N_CORES)
    qrtd = nc.declare_dram_parameter(
        "qrt", [ROWS_PER_CORE, D, IT_COLS], BF16, isOutput=False
    )
    zqd = nc.declare_dram_parameter(
        "zqd", [ROWS_PER_CORE, 128, L], BF16, isOutput=False
    )
    a2all = nc.declare_dram_parameter(
        "a2all", [128, ROWS_PER_CORE * 128], BF16, isOutput=False
    )
    uis2 = nc.declare_dram_parameter(
        "uis2", [ROWS_PER_CORE * L, I, 2], BF16, isOutput=True
    )
    uis8 = nc.declare_dram_parameter(
        "uis8", [ROWS_PER_CORE * L, I, 8], BF16, isOutput=True
    )
    pshs = nc.declare_dram_parameter(
        "pshs", [ROWS_PER_CORE * L, F], BF16, isOutput=True
    )
    pshs2 = nc.declare_dram_parameter(
        "pshs2", [ROWS_PER_CORE * L, 2, F], BF16, isOutput=True
    )
    out = nc.declare_dram_parameter(
        "out", [ROWS_PER_CORE, L, L], BF16, isOutput=True
    )

    with TileContext(nc) as tc:
        with (
            tc.tile_pool(name="const", bufs=1) as constp,
            tc.tile_pool(name="qrtp", bufs=ROWS_PER_CORE) as qrtp,
            tc.tile_pool(name="zqp", bufs=ROWS_PER_CORE) as zqp,
            tc.tile_pool(name="uip", bufs=2, space="PSUM") as uipp,
            tc.tile_pool(name="ps2", bufs=2, space="PSUM") as ps2p,
        ):
            a2t = constp.tile([128, ROWS_PER_CORE * 128], BF16)
            # pair-doubled l-major ui table:
            # uf2[f, j*512 + (i*16+c)*2 + k] = Ui_j[16f+i, c], k in {0,1}
            uf2 = constp.tile([F, ROWS_PER_CORE * 512], BF16)
            # l-partitioned psh for all rows: pshall[p, j*512 + lt*64 + f]
            pshall = constp.tile([128, ROWS_PER_CORE * 512], BF16)

            def body(_iv=None):
                eng = {"sp": nc.sync, "act": nc.scalar, "pool": nc.gpsimd}
                its = [qrtp.tile([D, IT_COLS], BF16, name=f"it{j}")
                       for j in range(ROWS_PER_CORE)]
                zqs = [zqp.tile([128, L], BF16, name=f"zq{j}")
                       for j in range(ROWS_PER_CORE)]
                for name, e in cfg["loads"]:
                    if name == "a2":
                        eng[e].dma_start(a2t[:], a2all[:])
                    elif name.startswith("it"):
                        j = int(name[2:])
                        eng[e].dma_start(its[j][:], qrtd[j])
                    else:
                        j = int(name[2:])
                        eng[e].dma_start(zqs[j][:], zqd[j])

                def emit_ui(j):
                    it = its[j]
                    # Ui, f-partitioned: stationary = q cols of instrument i,
                    # moving = R_i -> uip[f, 16i + c] = Ui[l = 16f + i, c]
                    qv = it[:, 0:1024].rearrange("d (f i) -> d i f", i=I)
                    uip = uipp.tile([F, 256], F32, name="uip")
                    for i in range(I):
                        nc.tensor.matmul(
                            uip[:, I * i : I * (i + 1)],
                            qv[:, i, :],
                            it[:, 1024 + I * i : 1024 + I * (i + 1)],
                        )
                    ov = (uf2[:, j * 512 : (j + 1) * 512]
                          .rearrange("f (ic k) -> f ic k", k=2))
                    iv = (uip[:].rearrange("f (ic o) -> f ic o", o=1)
                          .broadcast_to([F, 256, 2]))
                    nc.vector.tensor_copy(ov, iv)

                def emit_psh(j):
                    zq = zqs[j]
                    a2 = a2t[:, j * 128 : (j + 1) * 128]
                    # Psh: 32 matmuls into one [128, 512] PSUM tile.
                    # ps2[32g + r, 64lt + f] = Psh[l = lt*128 + 32g + r, f]
                    ps2 = ps2p.tile([128, 512], F32, name="ps2")
                    for lt in range(8):
                        for g in range(4):
                            fp = lt * 4 + g
                            nc.tensor.matmul(
                                ps2[32 * g : 32 * (g + 1),
                                    64 * lt : 64 * (lt + 1)],
                                zq[:, 32 * fp : 32 * (fp + 1)],
                                a2[:, 64 - 2 * fp : 128 - 2 * fp],
                                tile_position=(0, 32 * g),
                            )
                    nc.vector.tensor_copy(
                        pshall[:, j * 512 : (j + 1) * 512], ps2[:]
                    )

                outv = out[:].rearrange(
                    "j l (c f8 k) -> (j l) c f8 k", f8=8, k=8
                )

                def emit_uis2(j, e):
                    # uis2 store: src (f, (i c k)) -> uis2[l, c, k],
                    # l = j*L + 16f + i  (contiguous 512-elem runs both ways)
                    dst = (uis2[j * L : (j + 1) * L]
                           .rearrange("(f i) c k -> f (i c k)", i=I))
                    eng[e].dma_start(dst, uf2[:, j * 512 : (j + 1) * 512])

                def emit_casc(piece, e):
                    # uis8[l, c, 4m+k] = uis2[l, c, k] over a 512-l piece
                    sl = slice(piece * 512, (piece + 1) * 512)
                    srcc = (uis2[sl].rearrange("l c k -> (l c) k")
                            .rearrange("lc (o k) -> lc o k", o=1)
                            .broadcast_to([512 * I, 4, 2]))
                    dstc = uis8[sl].rearrange("l c (m k) -> (l c) m k", k=2)
                    eng[e].dma_start(dstc, srcc)

                def emit_oct(j0, nrows, o, e):
                    # ui write-expand, octet position o:
                    # out[l, c*64 + 8o + k] = uis8[l, c, k]
                    dst = outv[j0 * L : (j0 + nrows) * L][:, :, o, :]
                    srco = (uis8[j0 * L : (j0 + nrows) * L]
                            .rearrange("l c k -> (l c) k"))
                    eng[e].dma_start(dst, srco)

                def emit_pshs(j, e):
                    dst = (pshs[j * L : (j + 1) * L, :]
                           .rearrange("(lt p) f -> p lt f", p=128))
                    eng[e].dma_start(
                        dst,
                        pshall[:, j * 512 : (j + 1) * 512]
                        .rearrange("p (lt f) -> p lt f", f=F),
                    )

                def emit_dup(j, e):
                    # pshs2[l, k, f] = pshs[l, f], k in {0,1}: doubles the
                    # contiguous run per l to 128 elems for the accum.
                    sl = slice(j * L, (j + 1) * L)
                    srcd = (pshs[sl].rearrange("(l o) f -> l o f", o=1)
                            .broadcast_to([L, 2, F]))
                    eng[e].dma_start(pshs2[sl], srcd)

                def emit_acc(half):
                    # psh accum over a 512-l half-row, reading pshs2's
                    # 128-elem doubled runs -> 4096 SWDGE descriptors.
                    sl = slice(half * 512, (half + 1) * 512)
                    dst = (out[:]
                           .rearrange("j l (c2 kf) -> (j l) c2 kf", kf=128)
                           [half * 512 : (half + 1) * 512])
                    srca = (pshs2[sl].rearrange("l k f -> l (k f)")
                            .rearrange("l (o kf) -> l o kf", o=1)
                            .broadcast_to([512, 8, 128]))
                    nc.gpsimd.dma_start(dst, srca,
                                        accum_op=mybir.AluOpType.add)

                for step in cfg["emit"]:
                    kind = step[0]
                    if kind == "ui":
                        emit_ui(step[1])
                    elif kind == "psh":
                        emit_psh(step[1])
                    elif kind == "uis":
                        emit_uis2(step[1], step[2])
                    elif kind == "casc":
                        emit_casc(step[1], step[2])
                    elif kind == "oct":
                        emit_oct(step[1], step[2], step[3], step[4])
                    elif kind == "pshs":
                        emit_pshs(step[1], step[2])
                    elif kind == "dup":
                        emit_dup(step[1], step[2])
                    elif kind == "acc":
                        emit_acc(step[1])

            if loop_iters is None:
                body()
            else:
                with tc.For_i(0, loop_iters, 1) as _iv:
                    body(_iv)
    return nc


def make_in_maps(q, r_instrument, e_past):
    """Host-side sharding + table prep. Returns per-core input dicts."""
    q = np.asarray(q, dtype=np.float32)
    r_instrument = np.asarray(r_instrument, dtype=np.float32)
    e_past = np.asarray(e_past, dtype=np.float32)

    qT = q.reshape(32, L, D).transpose(0, 2, 1)  # (32, D, L)

    # zq[r, s*64+d, l] = qT[r, d, l] where (l//16) % 2 == s, else 0
    par = (np.arange(L) // I) % 2
    zq = np.zeros((32, 128, L), np.float32)
    for s in (0, 1):
        cols = par == s
        zq[:, s * 64 : (s + 1) * 64, cols] = qT[:, :, cols]

    # rt[h, d, 16i+c] = R[i, c, d, h]
    rt = r_instrument.transpose(3, 2, 0, 1).reshape(8, D, I * I)  # (8, D, 256)

    # a2[h, s*64+d, t] = a_h[(64 - t + s) % 65, d], t in [0, 128)
    a = np.concatenate(
        [e_past, np.full((1, D, 8), PAD_VAL, dtype=np.float32)], axis=0
    )  # (65, D, H)
    idx2 = (64 - np.arange(128)[None, :] + np.arange(2)[:, None]) % 65
    a2 = a[idx2]  # (2, 128, D, 8)
    a2 = a2.transpose(3, 0, 2, 1).reshape(8, 128, 128)  # (h, s*64+d, t)

    in_maps = []
    for k in range(N_CORES):
        rows = [ROWS_PER_CORE * k + j for j in range(ROWS_PER_CORE)]
        hs = [r % 8 for r in rows]
        its = []
        for r, h in zip(rows, hs):
            its.append(
                np.concatenate([qT[r], rt[h]], axis=1)[None]
            )  # (1, 64, 1280)
        a2c = np.concatenate([a2[h] for h in hs], axis=1)  # (128, 512)
        in_maps.append(
            {
                "qrt": np.ascontiguousarray(
                    np.concatenate(its, axis=0).astype(NP_BF16)
                ),
                "a2all": np.ascontiguousarray(a2c.astype(NP_BF16)),
                "zqd": np.ascontiguousarray(zq[rows].astype(NP_BF16)),
            }
        )
    return in_maps


def _get_program() -> bass.Bass:
    global _PROGRAM
    if _PROGRAM is None:
        _PROGRAM = build_program()
        if not _PROGRAM.is_finalized():
            _PROGRAM.finalize()
    return _PROGRAM


def kernel(q, r_instrument, e_past, flipped_masks=None, **_unused):
    in_maps = make_in_maps(q, r_instrument, e_past)
    res = run_bass_kernel_spmd(_get_program(), in_maps, list(range(N_CORES))).results
    blocks = []
    for k in range(N_CORES):
        dev = np.asarray(res[k]["out"], dtype=np.float32)  # (4, L, L) c-major
        # device n' = c*64 + f  ->  reference n = f*16 + c
        blocks.append(
            dev.reshape(ROWS_PER_CORE, L, I, F)
            .transpose(0, 1, 3, 2)
            .reshape(ROWS_PER_CORE, L, L)
        )
    return np.ascontiguousarray(np.concatenate(blocks, axis=0))


# revision 20
# speedup vs baseline: 1.1890x; 1.0860x over previous
"""Trainium2 Bass kernel for nn_BlockSelfAttentionModule.

Reference semantics (B=4, H=8, L=1024, I=16 instruments, F=64 frames, D=64):
  out[b*H+h, l, m] = q[l] . r_instrument[l%I, m%I, :, h]
                   + q[l] . a_h[(l//I - m//I) mod (F+1)]
  where a_h = concat(e_past[:, :, h], -111 pad row)   # (65, D)

Both bias terms factor through small per-row tables:
  Ui[l, c]  = q[l] . R_h[l%I, c]           (L x 16)
  Psh[l, f] = q[l] . a_h[(l//I - f) % 65]  (L x 64)
  out[l, f*16 + c] = Psh[l, f] + Ui[l, c]

Strategy (8 cores data-parallel over the 32 = B*H rows, 4 rows/core):
  The rank-2 outer-sum structure means the full (L, L) block never needs
  the classic materialize-and-store path (on-chip expansion + 25 us of
  SBUF->DRAM writes). Each core:
    1. computes Ui f-partitioned (16 PE matmuls, stationary = q columns of
       one instrument) and Psh l-partitioned (32 PE matmuls via the
       zero-padded zq / skewed a2 tables, K=128 covering 2 frames), with
       one DVE PSUM evacuation each,
    2. stores tiny staging tables to DRAM: uis2 (4L x 16 x 2, pair-doubled
       Ui) and pshs (4L x 64), then pshs2 (4L x 2 x 64, row-doubled via
       one broadcast DMA per row),
    3. expands them into the full c-major output (out[l, c*64+f]) with
       DRAM->DRAM DMAs whose DRAM-side dim0 is the whole l*c axis, so each
       costs ~500 ns regardless of bytes:
         - uis2 -> uis8 broadcast cascade (x4), then 16 octet-position
           writes out[l, c*64+8o+k] = uis8[l, c, k]  (HWDGE, big desc
           counts are fine on SP/ACT),
         - 8 half-row gpsimd CCE-add accumulates out[...] += psh, reading
           pshs2's 128-element doubled runs so each stays at 4096 SWDGE
           descriptors (8192+ wedges the device ring).
  All sources keep a real contiguous fastest dim (stride-0 only in middle
  dims) and all lowered AP dim counts stay under the 16-bit ISA field
  limit - both are hard neuronxcc requirements.
Host casts the returned bf16 blocks to f32 and permutes the last axis
back from c-major to the reference n = f*16 + c order.
"""

import numpy as np
import ml_dtypes

import concourse.bass as bass
import concourse.bacc as bacc
import concourse.mybir as mybir

from concourse.tile import TileContext
from concourse.bass_utils import run_bass_kernel_spmd

F32 = mybir.dt.float32
BF16 = mybir.dt.bfloat16
NP_BF16 = ml_dtypes.bfloat16

N_CORES = 8
ROWS_PER_CORE = 4  # (b*H + h) rows per core
L = 1024
D = 64
I = 16
F = 64
PAD_VAL = -111.0

IT_COLS = 1024 + 256  # qT | rt

_PROGRAM = None

# Schedule config, tuned against CoreSim: input-load engine assignment and
# the emission order of per-row compute, staging stores, and expansions.
DEFAULT_CFG = {
    "loads": [
        ("a2", "pool"),
        ("it0", "sp"), ("zq0", "act"),
        ("it1", "sp"), ("zq1", "act"),
        ("it2", "sp"), ("zq2", "act"),
        ("it3", "sp"), ("zq3", "act"),
    ],
    "emit": (
        [("ui", 0), ("psh", 0), ("ui", 1), ("psh", 1),
         ("ui", 2), ("psh", 2), ("ui", 3), ("psh", 3),
         ("uis", 0, "act"), ("casc", 0, "sp"), ("casc", 1, "act"),
         ("pshs", 0, "sp"), ("dup", 0, "pool"),
         ("uis", 1, "act"), ("casc", 2, "sp"), ("casc", 3, "act"),
         ("pshs", 1, "sp"), ("dup", 1, "pool")]
        + [("oct", 0, 2, o, ("sp", "act")[o % 2]) for o in range(8)]
        + [("acc", 0), ("acc", 1), ("acc", 2), ("acc", 3)]
        + [("uis", 2, "act"), ("casc", 4, "sp"), ("casc", 5, "act"),
           ("pshs", 2, "sp"), ("dup", 2, "pool"),
           ("uis", 3, "act"), ("casc", 6, "sp"), ("casc", 7, "act"),
           ("pshs", 3, "sp"), ("dup", 3, "pool")]
        + [("oct", 2, 2, o, ("sp", "act")[o % 2]) for o in range(8)]
        + [("acc", 4), ("acc", 5), ("acc", 6), ("acc", 7)]
    ),
}


def build_program(loop_iters: int | None = None, cfg=None) -> bass.Bass:
    """loop_iters: device-side repeat loop for benchmarking only."""
    cfg = cfg or DEFAULT_CFG
    nc = bacc.Bacc("TRN2", debug=False, num_devices=N_CORES)
    qrtd = nc.declare_dram_parameter(
        "qrt", [ROWS_PER_CORE, D, IT_COLS], BF16, isOutput=False
    )
    zqd = nc.declare_dram_parameter(
        "zqd", [ROWS_PER_CORE, 128, L], BF16, isOutput=False
    )
    a2all = nc.declare_dram_parameter(
        "a2all", [128, ROWS_PER_CORE * 128], BF16, isOutput=False
    )
    uis2 = nc.declare_dram_parameter(
        "uis2", [ROWS_PER_CORE * L, I, 2], BF16, isOutput=True
    )
    uis8 = nc.declare_dram_parameter(
        "uis8", [ROWS_PER_CORE * L, I, 8], BF16, isOutput=True
    )
    pshs = nc.declare_dram_parameter(
        "pshs", [ROWS_PER_CORE * L, F], BF16, isOutput=True
    )
    pshs2 = nc.declare_dram_parameter(
        "pshs2", [ROWS_PER_CORE * L, 2, F], BF16, isOutput=True
    )
    out = nc.declare_dram_parameter(
        "out", [ROWS_PER_CORE, L, L], BF16, isOutput=True
    )

    with TileContext(nc) as tc:
        with (
            tc.tile_pool(name="const", bufs=1) as constp,
            tc.tile_pool(name="qrtp", bufs=ROWS_PER_CORE) as qrtp,
            tc.tile_pool(name="zqp", bufs=ROWS_PER_CORE) as zqp,
            tc.tile_pool(name="uip", bufs=2, space="PSUM") as uipp,
            tc.tile_pool(name="ps2", bufs=2, space="PSUM") as ps2p,
        ):
            a2t = constp.tile([128, ROWS_PER_CORE * 128], BF16)
            # pair-doubled l-major ui table:
            # uf2[f, j*512 + (i*16+c)*2 + k] = Ui_j[16f+i, c], k in {0,1}
            uf2 = constp.tile([F, ROWS_PER_CORE * 512], BF16)
            # l-partitioned psh for all rows: pshall[p, j*512 + lt*64 + f]
            pshall = constp.tile([128, ROWS_PER_CORE * 512], BF16)

            def body(_iv=None):
                eng = {"sp": nc.sync, "act": nc.scalar, "pool": nc.gpsimd}
                its = [qrtp.tile([D, IT_COLS], BF16, name=f"it{j}")
                       for j in range(ROWS_PER_CORE)]
                zqs = [zqp.tile([128, L], BF16, name=f"zq{j}")
                       for j in range(ROWS_PER_CORE)]
                for name, e in cfg["loads"]:
                    if name == "a2":
                        eng[e].dma_start(a2t[:], a2all[:])
                    elif name.startswith("it"):
                        j = int(name[2:])
                        eng[e].dma_start(its[j][:], qrtd[j])
                    else:
                        j = int(name[2:])
                        eng[e].dma_start(zqs[j][:], zqd[j])

                def emit_ui(j):
                    it = its[j]
                    # Ui, f-partitioned: stationary = q cols of instrument i,
                    # moving = R_i -> uip[f, 16i + c] = Ui[l = 16f + i, c]
                    qv = it[:, 0:1024].rearrange("d (f i) -> d i f", i=I)
                    uip = uipp.tile([F, 256], F32, name="uip")
                    for i in range(I):
                        nc.tensor.matmul(
                            uip[:, I * i : I * (i + 1)],
                            qv[:, i, :],
                            it[:, 1024 + I * i : 1024 + I * (i + 1)],
                        )
                    ov = (uf2[:, j * 512 : (j + 1) * 512]
                          .rearrange("f (ic k) -> f ic k", k=2))
                    iv = (uip[:].rearrange("f (ic o) -> f ic o", o=1)
                          .broadcast_to([F, 256, 2]))
                    nc.vector.tensor_copy(ov, iv)

                def emit_psh(j):
                    zq = zqs[j]
                    a2 = a2t[:, j * 128 : (j + 1) * 128]
                    # Psh: 32 matmuls into one [128, 512] PSUM tile.
                    # ps2[32g + r, 64lt + f] = Psh[l = lt*128 + 32g + r, f]
                    ps2 = ps2p.tile([128, 512], F32, name="ps2")
                    for lt in range(8):
                        for g in range(4):
                            fp = lt * 4 + g
                            nc.tensor.matmul(
                                ps2[32 * g : 32 * (g + 1),
                                    64 * lt : 64 * (lt + 1)],
                                zq[:, 32 * fp : 32 * (fp + 1)],
                                a2[:, 64 - 2 * fp : 128 - 2 * fp],
                                tile_position=(0, 32 * g),
                            )
                    nc.vector.tensor_copy(
                        pshall[:, j * 512 : (j + 1) * 512], ps2[:]
                    )

                outv = out[:].rearrange(
                    "j l (c f8 k) -> (j l) c f8 k", f8=8, k=8
                )

                def emit_uis2(j, e):
                    # uis2 store: src (f, (i c k)) -> uis2[l, c, k],
                    # l = j*L + 16f + i  (contiguous 512-elem runs both ways)
                    dst = (uis2[j * L : (j + 1) * L]
                           .rearrange("(f i) c k -> f (i c k)", i=I))
                    eng[e].dma_start(dst, uf2[:, j * 512 : (j + 1) * 512])

                def emit_casc(piece, e):
                    # uis8[l, c, 4m+k] = uis2[l, c, k] over a 512-l piece
                    sl = slice(piece * 512, (piece + 1) * 512)
                    srcc = (uis2[sl].rearrange("l c k -> (l c) k")
                            .rearrange("lc (o k) -> lc o k", o=1)
                            .broadcast_to([512 * I, 4, 2]))
                    dstc = uis8[sl].rearrange("l c (m k) -> (l c) m k", k=2)
                    eng[e].dma_start(dstc, srcc)

                def emit_oct(j0, nrows, o, e):
                    # ui write-expand, octet position o:
                    # out[l, c*64 + 8o + k] = uis8[l, c, k]
                    dst = outv[j0 * L : (j0 + nrows) * L][:, :, o, :]
                    srco = (uis8[j0 * L : (j0 + nrows) * L]
                            .rearrange("l c k -> (l c) k"))
                    eng[e].dma_start(dst, srco)

                def emit_pshs(j, e):
                    dst = (pshs[j * L : (j + 1) * L, :]
                           .rearrange("(lt p) f -> p lt f", p=128))
                    eng[e].dma_start(
                        dst,
                        pshall[:, j * 512 : (j + 1) * 512]
                        .rearrange("p (lt f) -> p lt f", f=F),
                    )

                def emit_dup(j, e):
                    # pshs2[l, k, f] = pshs[l, f], k in {0,1}: doubles the
                    # contiguous run per l to 128 elems for the accum.
                    sl = slice(j * L, (j + 1) * L)
                    srcd = (pshs[sl].rearrange("(l o) f -> l o f", o=1)
                            .broadcast_to([L, 2, F]))
                    eng[e].dma_start(pshs2[sl], srcd)

                def emit_acc(half):
                    # psh accum over a 512-l half-row, reading pshs2's
                    # 128-elem doubled runs -> 4096 SWDGE descriptors.
                    sl = slice(half * 512, (half + 1) * 512)
                    dst = (out[:]
                           .rearrange("j l (c2 kf) -> (j l) c2 kf", kf=128)
                           [half * 512 : (half + 1) * 512])
                    srca = (pshs2[sl].rearrange("l k f -> l (k f)")
                            .rearrange("l (o kf) -> l o kf", o=1)
                            .broadcast_to([512, 8, 128]))
                    nc.gpsimd.dma_start(dst, srca,
                                        accum_op=mybir.AluOpType.add)

                for step in cfg["emit"]:
                    kind = step[0]
                    if kind == "ui":
                        emit_ui(step[1])
                    elif kind == "psh":
                        emit_psh(step[1])
                    elif kind == "uis":
                        emit_uis2(step[1], step[2])
                    elif kind == "casc":
                        emit_casc(step[1], step[2])
                    elif kind == "oct":
                        emit_oct(step[1], step[2], step[3], step[4])
                    elif kind == "pshs":
                        emit_pshs(step[1], step[2])
                    elif kind == "dup":
                        emit_dup(step[1], step[2])
                    elif kind == "acc":
                        emit_acc(step[1])

            if loop_iters is None:
                body()
            else:
                with tc.For_i(0, loop_iters, 1) as _iv:
                    body(_iv)
    return nc


def make_in_maps(q, r_instrument, e_past):
    """Host-side sharding + table prep. Returns per-core input dicts."""
    q = np.asarray(q, dtype=np.float32)
    r_instrument = np.asarray(r_instrument, dtype=np.float32)
    e_past = np.asarray(e_past, dtype=np.float32)

    qT = q.reshape(32, L, D).transpose(0, 2, 1)  # (32, D, L)

    # zq[r, s*64+d, l] = qT[r, d, l] where (l//16) % 2 == s, else 0
    par = (np.arange(L) // I) % 2
    zq = np.zeros((32, 128, L), np.float32)
    for s in (0, 1):
        cols = par == s
        zq[:, s * 64 : (s + 1) * 64, cols] = qT[:, :, cols]

    # rt[h, d, 16i+c] = R[i, c, d, h]
    rt = r_instrument.transpose(3, 2, 0, 1).reshape(8, D, I * I)  # (8, D, 256)

    # a2[h, s*64+d, t] = a_h[(64 - t + s) % 65, d], t in [0, 128)
    a = np.concatenate(
        [e_past, np.full((1, D, 8), PAD_VAL, dtype=np.float32)], axis=0
    )  # (65, D, H)
    idx2 = (64 - np.arange(128)[None, :] + np.arange(2)[:, None]) % 65
    a2 = a[idx2]  # (2, 128, D, 8)
    a2 = a2.transpose(3, 0, 2, 1).reshape(8, 128, 128)  # (h, s*64+d, t)

    in_maps = []
    for k in range(N_CORES):
        rows = [ROWS_PER_CORE * k + j for j in range(ROWS_PER_CORE)]
        hs = [r % 8 for r in rows]
        its = []
        for r, h in zip(rows, hs):
            its.append(
                np.concatenate([qT[r], rt[h]], axis=1)[None]
            )  # (1, 64, 1280)
        a2c = np.concatenate([a2[h] for h in hs], axis=1)  # (128, 512)
        in_maps.append(
            {
                "qrt": np.ascontiguousarray(
                    np.concatenate(its, axis=0).astype(NP_BF16)
                ),
                "a2all": np.ascontiguousarray(a2c.astype(NP_BF16)),
                "zqd": np.ascontiguousarray(zq[rows].astype(NP_BF16)),
            }
        )
    return in_maps


def _get_program() -> bass.Bass:
    global _PROGRAM
    if _PROGRAM is None:
        _PROGRAM = build_program()
        if not _PROGRAM.is_finalized():
            _PROGRAM.finalize()
    return _PROGRAM


def kernel(q, r_instrument, e_past, flipped_masks=None, **_unused):
    in_maps = make_in_maps(q, r_instrument, e_past)
    res = run_bass_kernel_spmd(_get_program(), in_maps, list(range(N_CORES))).results
    blocks = []
    for k in range(N_CORES):
        dev = np.asarray(res[k]["out"], dtype=np.float32)  # (4, L, L) c-major
        # device n' = c*64 + f  ->  reference n = f*16 + c
        blocks.append(
            dev.reshape(ROWS_PER_CORE, L, I, F)
            .transpose(0, 1, 3, 2)
            .reshape(ROWS_PER_CORE, L, L)
        )
    return np.ascontiguousarray(np.concatenate(blocks, axis=0))


# revision 23
# speedup vs baseline: 1.2469x; 1.0487x over previous
"""Trainium2 Bass kernel for nn_BlockSelfAttentionModule.

Reference semantics (B=4, H=8, L=1024, I=16 instruments, F=64 frames, D=64):
  out[b*H+h, l, m] = q[l] . r_instrument[l%I, m%I, :, h]
                   + q[l] . a_h[(l//I - m//I) mod (F+1)]
  where a_h = concat(e_past[:, :, h], -111 pad row)   # (65, D)

Both bias terms factor through small per-row tables:
  Ui[l, c]  = q[l] . R_h[l%I, c]           (L x 16)
  Psh[l, f] = q[l] . a_h[(l//I - f) % 65]  (L x 64)
  out[l, f*16 + c] = Psh[l, f] + Ui[l, c]

Strategy (8 cores data-parallel over the 32 = B*H rows, 4 rows/core):
  The rank-2 outer-sum structure means the full (L, L) block never needs
  the classic materialize-and-store path (on-chip expansion + 25 us of
  SBUF->DRAM writes). Each core:
    1. computes Ui f-partitioned (16 PE matmuls, stationary = q columns of
       one instrument) and Psh l-partitioned (32 PE matmuls via the
       zero-padded zq / skewed a2 tables, K=128 covering 2 frames), with
       one DVE PSUM evacuation each,
    2. stores tiny staging tables to DRAM: uis2 (4L x 16 x 2, pair-doubled
       Ui) and pshs (4L x 64), then pshs2 (4L x 2 x 64, row-doubled via
       one broadcast DMA per row),
    3. expands them into the full c-major output (out[l, c*64+f]) with
       DRAM->DRAM DMAs whose DRAM-side dim0 is the whole l*c axis, so each
       costs ~500 ns regardless of bytes:
         - uis2 -> uis8 broadcast cascade (x4), then 16 octet-position
           writes out[l, c*64+8o+k] = uis8[l, c, k]  (HWDGE, big desc
           counts are fine on SP/ACT),
         - 8 half-row gpsimd CCE-add accumulates out[...] += psh, reading
           pshs2's 128-element doubled runs so each stays at 4096 SWDGE
           descriptors (8192+ wedges the device ring).
  All sources keep a real contiguous fastest dim (stride-0 only in middle
  dims) and all lowered AP dim counts stay under the 16-bit ISA field
  limit - both are hard neuronxcc requirements.
Host casts the returned bf16 blocks to f32 and permutes the last axis
back from c-major to the reference n = f*16 + c order.
"""

import numpy as np
import ml_dtypes

import concourse.bass as bass
import concourse.bacc as bacc
import concourse.mybir as mybir

from concourse.tile import TileContext
from concourse.bass_utils import run_bass_kernel_spmd

F32 = mybir.dt.float32
BF16 = mybir.dt.bfloat16
NP_BF16 = ml_dtypes.bfloat16

N_CORES = 8
ROWS_PER_CORE = 4  # (b*H + h) rows per core
L = 1024
D = 64
I = 16
F = 64
PAD_VAL = -111.0

IT_COLS = 1024 + 256  # qT | rt

_PROGRAM = None

# Schedule config, tuned against CoreSim: input-load engine assignment and
# the emission order of per-row compute, staging stores, and expansions.
DEFAULT_CFG = {
    "loads": [
        ("a2", "pool"),
        ("it0", "sp"), ("zq0", "act"),
        ("it1", "sp"), ("zq1", "act"),
        ("it2", "sp"), ("zq2", "act"),
        ("it3", "sp"), ("zq3", "act"),
    ],
    "emit": (
        [("ui", 0), ("psh", 0), ("ui", 1), ("psh", 1),
         ("ui", 2), ("psh", 2), ("ui", 3), ("psh", 3),
         ("uis", 0, "act"), ("casc", 0, "sp"), ("casc", 1, "act"),
         ("casc2", 0, "sp"), ("pshs", 0, "pool"), ("dup", 0, "sp")]
        + [("oct", 0, 1, o, ("sp", "act")[o % 2]) for o in range(4)]
        + [("acc", 0),
           ("uis", 1, "act"), ("casc", 2, "sp"), ("casc", 3, "act"),
           ("casc2", 1, "act"), ("pshs", 1, "pool"), ("dup", 1, "sp")]
        + [("oct", 1, 1, o, ("sp", "act")[o % 2]) for o in range(4)]
        + [("acc", 1),
           ("uis", 2, "act"), ("casc", 4, "sp"), ("casc", 5, "act"),
           ("casc2", 2, "sp"), ("pshs", 2, "pool"), ("dup", 2, "sp")]
        + [("oct", 2, 1, o, ("sp", "act")[o % 2]) for o in range(4)]
        + [("acc", 2),
           ("uis", 3, "act"), ("casc", 6, "sp"), ("casc", 7, "act"),
           ("casc2", 3, "act"), ("pshs", 3, "pool"), ("dup", 3, "sp")]
        + [("oct", 3, 1, o, ("sp", "act")[o % 2]) for o in range(4)]
        + [("acc", 3)]
    ),
}


def build_program(loop_iters: int | None = None, cfg=None) -> bass.Bass:
    """loop_iters: device-side repeat loop for benchmarking only."""
    cfg = cfg or DEFAULT_CFG
    nc = bacc.Bacc("TRN2", debug=False, num_devices=N_CORES)
    qrtd = nc.declare_dram_parameter(
        "qrt", [ROWS_PER_CORE, D, IT_COLS], BF16, isOutput=False
    )
    zqd = nc.declare_dram_parameter(
        "zqd", [ROWS_PER_CORE, 128, L], BF16, isOutput=False
    )
    a2all = nc.declare_dram_parameter(
        "a2all", [128, ROWS_PER_CORE * 128], BF16, isOutput=False
    )
    uis2 = nc.declare_dram_parameter(
        "uis2", [ROWS_PER_CORE * L, I, 2], BF16, isOutput=True
    )
    uis8 = nc.declare_dram_parameter(
        "uis8", [ROWS_PER_CORE * L, I, 8], BF16, isOutput=True
    )
    uis16 = nc.declare_dram_parameter(
        "uis16", [ROWS_PER_CORE * L, I, 16], BF16, isOutput=True
    )
    pshs = nc.declare_dram_parameter(
        "pshs", [ROWS_PER_CORE * L, F], BF16, isOutput=True
    )
    pshs4 = nc.declare_dram_parameter(
        "pshs4", [ROWS_PER_CORE * L, 4, F], BF16, isOutput=True
    )
    out = nc.declare_dram_parameter(
        "out", [ROWS_PER_CORE, L, L], BF16, isOutput=True
    )

    with TileContext(nc) as tc:
        with (
            tc.tile_pool(name="const", bufs=1) as constp,
            tc.tile_pool(name="qrtp", bufs=ROWS_PER_CORE) as qrtp,
            tc.tile_pool(name="zqp", bufs=ROWS_PER_CORE) as zqp,
            tc.tile_pool(name="uip", bufs=2, space="PSUM") as uipp,
            tc.tile_pool(name="ps2", bufs=2, space="PSUM") as ps2p,
        ):
            a2t = constp.tile([128, ROWS_PER_CORE * 128], BF16)
            # pair-doubled l-major ui table:
            # uf2[f, j*512 + (i*16+c)*2 + k] = Ui_j[16f+i, c], k in {0,1}
            uf2 = constp.tile([F, ROWS_PER_CORE * 512], BF16)
            # l-partitioned psh for all rows: pshall[p, j*512 + lt*64 + f]
            pshall = constp.tile([128, ROWS_PER_CORE * 512], BF16)

            def body(_iv=None):
                eng = {"sp": nc.sync, "act": nc.scalar, "pool": nc.gpsimd}
                its = [qrtp.tile([D, IT_COLS], BF16, name=f"it{j}")
                       for j in range(ROWS_PER_CORE)]
                zqs = [zqp.tile([128, L], BF16, name=f"zq{j}")
                       for j in range(ROWS_PER_CORE)]
                for name, e in cfg["loads"]:
                    if name == "a2":
                        eng[e].dma_start(a2t[:], a2all[:])
                    elif name.startswith("it"):
                        j = int(name[2:])
                        eng[e].dma_start(its[j][:], qrtd[j])
                    else:
                        j = int(name[2:])
                        eng[e].dma_start(zqs[j][:], zqd[j])

                def emit_ui(j):
                    it = its[j]
                    # Ui, f-partitioned: stationary = q cols of instrument i,
                    # moving = R_i -> uip[f, 16i + c] = Ui[l = 16f + i, c]
                    qv = it[:, 0:1024].rearrange("d (f i) -> d i f", i=I)
                    uip = uipp.tile([F, 256], F32, name="uip")
                    for i in range(I):
                        nc.tensor.matmul(
                            uip[:, I * i : I * (i + 1)],
                            qv[:, i, :],
                            it[:, 1024 + I * i : 1024 + I * (i + 1)],
                        )
                    ov = (uf2[:, j * 512 : (j + 1) * 512]
                          .rearrange("f (ic k) -> f ic k", k=2))
                    iv = (uip[:].rearrange("f (ic o) -> f ic o", o=1)
                          .broadcast_to([F, 256, 2]))
                    nc.vector.tensor_copy(ov, iv)

                def emit_psh(j):
                    zq = zqs[j]
                    a2 = a2t[:, j * 128 : (j + 1) * 128]
                    # Psh: 32 matmuls into one [128, 512] PSUM tile.
                    # ps2[32g + r, 64lt + f] = Psh[l = lt*128 + 32g + r, f]
                    ps2 = ps2p.tile([128, 512], F32, name="ps2")
                    for lt in range(8):
                        for g in range(4):
                            fp = lt * 4 + g
                            nc.tensor.matmul(
                                ps2[32 * g : 32 * (g + 1),
                                    64 * lt : 64 * (lt + 1)],
                                zq[:, 32 * fp : 32 * (fp + 1)],
                                a2[:, 64 - 2 * fp : 128 - 2 * fp],
                                tile_position=(0, 32 * g),
                            )
                    nc.vector.tensor_copy(
                        pshall[:, j * 512 : (j + 1) * 512], ps2[:]
                    )

                outv = out[:].rearrange(
                    "j l (c f4 k) -> (j l) c f4 k", f4=4, k=16
                )

                def emit_uis2(j, e):
                    # uis2 store: src (f, (i c k)) -> uis2[l, c, k],
                    # l = j*L + 16f + i  (contiguous 512-elem runs both ways)
                    dst = (uis2[j * L : (j + 1) * L]
                           .rearrange("(f i) c k -> f (i c k)", i=I))
                    eng[e].dma_start(dst, uf2[:, j * 512 : (j + 1) * 512])

                def emit_casc(piece, e):
                    # uis8[l, c, 4m+k] = uis2[l, c, k] over a 512-l piece
                    sl = slice(piece * 512, (piece + 1) * 512)
                    srcc = (uis2[sl].rearrange("l c k -> (l c) k")
                            .rearrange("lc (o k) -> lc o k", o=1)
                            .broadcast_to([512 * I, 4, 2]))
                    dstc = uis8[sl].rearrange("l c (m k) -> (l c) m k", k=2)
                    eng[e].dma_start(dstc, srcc)

                def emit_casc2(j, e):
                    # uis16[l, c, 8m+k] = uis8[l, c, k] over one row (1024 l)
                    sl = slice(j * L, (j + 1) * L)
                    srcc = (uis8[sl].rearrange("l c k -> (l c) k")
                            .rearrange("lc (o k) -> lc o k", o=1)
                            .broadcast_to([L * I, 2, 8]))
                    dstc = uis16[sl].rearrange("l c (m k) -> (l c) m k", k=8)
                    eng[e].dma_start(dstc, srcc)

                def emit_oct(j0, nrows, o, e):
                    # ui write-expand, 16-wide position o:
                    # out[l, c*64 + 16o + k] = uis16[l, c, k]
                    dst = outv[j0 * L : (j0 + nrows) * L][:, :, o, :]
                    srco = (uis16[j0 * L : (j0 + nrows) * L]
                            .rearrange("l c k -> (l c) k"))
                    eng[e].dma_start(dst, srco)

                def emit_pshs(j, e):
                    dst = (pshs[j * L : (j + 1) * L, :]
                           .rearrange("(lt p) f -> p lt f", p=128))
                    eng[e].dma_start(
                        dst,
                        pshall[:, j * 512 : (j + 1) * 512]
                        .rearrange("p (lt f) -> p lt f", f=F),
                    )

                def emit_dup(j, e):
                    # pshs4[l, k, f] = pshs[l, f], k in 0..3: quadruples the
                    # contiguous run per l to 256 elems for the accum.
                    sl = slice(j * L, (j + 1) * L)
                    srcd = (pshs[sl].rearrange("(l o) f -> l o f", o=1)
                            .broadcast_to([L, 4, F]))
                    eng[e].dma_start(pshs4[sl], srcd)

                def emit_acc(j):
                    # psh accum over a full row, reading pshs4's 256-elem
                    # quadrupled runs -> 4096 SWDGE descriptors.
                    sl = slice(j * L, (j + 1) * L)
                    dst = (out[:]
                           .rearrange("j l (c4 kf) -> (j l) c4 kf", kf=256)
                           [j * L : (j + 1) * L])
                    srca = (pshs4[sl].rearrange("l k f -> l (k f)")
                            .rearrange("l (o kf) -> l o kf", o=1)
                            .broadcast_to([L, 4, 256]))
                    nc.gpsimd.dma_start(dst, srca,
                                        accum_op=mybir.AluOpType.add)

                for step in cfg["emit"]:
                    kind = step[0]
                    if kind == "ui":
                        emit_ui(step[1])
                    elif kind == "psh":
                        emit_psh(step[1])
                    elif kind == "uis":
                        emit_uis2(step[1], step[2])
                    elif kind == "casc":
                        emit_casc(step[1], step[2])
                    elif kind == "casc2":
                        emit_casc2(step[1], step[2])
                    elif kind == "oct":
                        emit_oct(step[1], step[2], step[3], step[4])
                    elif kind == "pshs":
                        emit_pshs(step[1], step[2])
                    elif kind == "dup":
                        emit_dup(step[1], step[2])
                    elif kind == "acc":
                        emit_acc(step[1])

            if loop_iters is None:
                body()
            else:
                with tc.For_i(0, loop_iters, 1) as _iv:
                    body(_iv)
    return nc


def make_in_maps(q, r_instrument, e_past):
    """Host-side sharding + table prep. Returns per-core input dicts."""
    q = np.asarray(q, dtype=np.float32)
    r_instrument = np.asarray(r_instrument, dtype=np.float32)
    e_past = np.asarray(e_past, dtype=np.float32)

    qT = q.reshape(32, L, D).transpose(0, 2, 1)  # (32, D, L)

    # zq[r, s*64+d, l] = qT[r, d, l] where (l//16) % 2 == s, else 0
    par = (np.arange(L) // I) % 2
    zq = np.zeros((32, 128, L), np.float32)
    for s in (0, 1):
        cols = par == s
        zq[:, s * 64 : (s + 1) * 64, cols] = qT[:, :, cols]

    # rt[h, d, 16i+c] = R[i, c, d, h]
    rt = r_instrument.transpose(3, 2, 0, 1).reshape(8, D, I * I)  # (8, D, 256)

    # a2[h, s*64+d, t] = a_h[(64 - t + s) % 65, d], t in [0, 128)
    a = np.concatenate(
        [e_past, np.full((1, D, 8), PAD_VAL, dtype=np.float32)], axis=0
    )  # (65, D, H)
    idx2 = (64 - np.arange(128)[None, :] + np.arange(2)[:, None]) % 65
    a2 = a[idx2]  # (2, 128, D, 8)
    a2 = a2.transpose(3, 0, 2, 1).reshape(8, 128, 128)  # (h, s*64+d, t)

    in_maps = []
    for k in range(N_CORES):
        rows = [ROWS_PER_CORE * k + j for j in range(ROWS_PER_CORE)]
        hs = [r % 8 for r in rows]
        its = []
        for r, h in zip(rows, hs):
            its.append(
                np.concatenate([qT[r], rt[h]], axis=1)[None]
            )  # (1, 64, 1280)
        a2c = np.concatenate([a2[h] for h in hs], axis=1)  # (128, 512)
        in_maps.append(
            {
                "qrt": np.ascontiguousarray(
                    np.concatenate(its, axis=0).astype(NP_BF16)
                ),
                "a2all": np.ascontiguousarray(a2c.astype(NP_BF16)),
                "zqd": np.ascontiguousarray(zq[rows].astype(NP_BF16)),
            }
        )
    return in_maps


def _get_program() -> bass.Bass:
    global _PROGRAM
    if _PROGRAM is None:
        _PROGRAM = build_program()
        if not _PROGRAM.is_finalized():
            _PROGRAM.finalize()
    return _PROGRAM


def kernel(q, r_instrument, e_past, flipped_masks=None, **_unused):
    in_maps = make_in_maps(q, r_instrument, e_past)
    res = run_bass_kernel_spmd(_get_program(), in_maps, list(range(N_CORES))).results
    blocks = []
    for k in range(N_CORES):
        dev = np.asarray(res[k]["out"], dtype=np.float32)  # (4, L, L) c-major
        # device n' = c*64 + f  ->  reference n = f*16 + c
        blocks.append(
            dev.reshape(ROWS_PER_CORE, L, I, F)
            .transpose(0, 1, 3, 2)
            .reshape(ROWS_PER_CORE, L, L)
        )
    return np.ascontiguousarray(np.concatenate(blocks, axis=0))
